# revision 17
# baseline (speedup 1.0000x reference)
"""Causal self-attention (B=4, T=1024, C=1024, H=16) on 8 Trainium2 cores.

Sharding: hybrid batch x head-group. Core c owns batch c//2 and head-group
c%2 (8 heads = 4 pairs of 2). Each core computes its batch's q/k/v slice,
causal attention for its 8 heads, and a partial projection (contraction
over its 512 rows of w_proj). Host sums the 2 partials per batch + b_proj.

v2 structure (vs the f32r baseline):
  - All matmul operands are bf16 (full PE rate at any moving width; f32r
    drops to 1/4 rate below N=256 and doubles SBUF/DMA traffic).
  - V is produced directly in [keys, dv] layout by a separate pass with
    xT tiles stationary (out = xT_tile.T @ wv), eliminating all PE
    transposes.  Ones columns for the softmax denominator are pre-filled
    in V_sb; V bias is added during the PSUM evacuation.
  - S for the two heads (row-packed via tile_position) accumulates into
    one 2-bank PSUM tile, so exp() runs as a single ACT instruction per
    key block (half the ACT instruction count).
  - q/k bias is applied by the ACT evacuation (identity+bias), V bias by
    the DVE evacuation; no standalone bias instructions.
  - Softmax normalization: reciprocal rows -> one 128-row PE broadcast
    (A rows 0:64, B rows 64:128) per query chunk.

Rel err budget: bf16 inputs give ~0.3% worst-case output error vs the
2e-2 gate.
"""

import numpy as np
import concourse.bass as bass
import concourse.mybir as mybir
import concourse.tile as tile
from concourse.bass import ts
from concourse.bass_utils import run_bass_kernel_spmd

F32 = mybir.dt.float32
F32R = mybir.dt.float32r
BF16 = mybir.dt.bfloat16
AF = mybir.ActivationFunctionType
FP8 = mybir.dt.float8e4
DR = mybir.MatmulPerfMode.DoubleRow
WSCALE = 64.0

B, T, C, H = 4, 1024, 1024, 16
D = C // H            # 64
NCORES = 8
NPAIR = 4             # head pairs per core
NEG = -30000.0
CT = C // 128         # 8 contraction tiles
KB = T // 128         # 8 key blocks
QC = T // 512         # 2 query chunks

_nc_cache = {}


def _split_sync_waits(nc):
    """This walrus build accepts exactly one sem-wait per instruction; move
    overflow waits onto fresh same-engine NoOps inserted just before."""
    n = 0
    for fn in nc.m.functions:
        for blk in fn.blocks:
            new_insts = []
            for inst in blk.instructions:
                si = getattr(inst, "sync_info", None)
                waits = list(si.on_wait) if si is not None and si.on_wait else []
                if len(waits) > 1:
                    for w in waits[1:]:
                        n += 1
                        new_insts.append(mybir.InstNoOp(
                            name=f"waitfix-{n}-{id(inst) & 0xffff}",
                            sync_info=mybir.SyncInfo(on_wait=[w], on_update=[]),
                            bass_nofuse=True,
                            engine=inst.engine,
                        ))
                    si.on_wait = waits[:1]
                new_insts.append(inst)
            blk.instructions[:] = new_insts
    return n


def build_nc(reps=1):
    nc = bass.Bass()
    x8_d = nc.dram_tensor("x8", [C // 2, 2 * T], FP8, kind="ExternalInput")
    wqk8_d = nc.dram_tensor("wqk8", [C // 2, 2 * 8 * 128], FP8,
                            kind="ExternalInput")
    xT_d = nc.dram_tensor("xTb", [C, T], BF16, kind="ExternalInput")
    battnqk_d = nc.dram_tensor("battnqk", [128, 8], F32, kind="ExternalInput")
    wv_d = nc.dram_tensor("wv", [C, 512], BF16, kind="ExternalInput")
    bvb_d = nc.dram_tensor("bvb", [128, 512], BF16, kind="ExternalInput")
    wproj_d = nc.dram_tensor("wproj", [NPAIR * 128, C], BF16, kind="ExternalInput")
    idb_d = nc.dram_tensor("idb", [128, 128], BF16, kind="ExternalInput")
    mask_d = nc.dram_tensor("mask", [128, 128], BF16, kind="ExternalInput")
    onesrow_d = nc.dram_tensor("onesrow", [1, 64], BF16, kind="ExternalInput")
    onesb_d = nc.dram_tensor("onesb", [128, 1], BF16, kind="ExternalInput")
    out_d = nc.dram_tensor("partial", [T, C], F32, kind="ExternalOutput")

    with tile.TileContext(nc) as tc:
        with tc.tile_pool(name="const", bufs=1) as cpool, \
             tc.tile_pool(name="wp", bufs=1) as wpool, \
             tc.tile_pool(name="xp", bufs=2) as xpool, \
             tc.tile_pool(name="vp", bufs=2) as vpool, \
             tc.tile_pool(name="qk", bufs=2) as qkpool, \
             tc.tile_pool(name="pp", bufs=4) as ppool, \
             tc.tile_pool(name="yp", bufs=8) as ypool, \
             tc.tile_pool(name="op", bufs=4) as opool, \
             tc.tile_pool(name="ps", bufs=1, space="PSUM") as ps:

            wqk8 = wpool.tile([128, 4, 2, 8, 128], FP8)
            wv = wpool.tile([128, CT, 512], BF16)
            wproj = wpool.tile([128, NPAIR, 2, 512], BF16)
            battnqk = cpool.tile([128, 8], F32)
            bvb = cpool.tile([128, 8, 64], BF16)
            idb = cpool.tile([128, 128], BF16)
            maskb = cpool.tile([128, 128], BF16)
            onesrow = cpool.tile([1, 64], BF16)
            onesb = cpool.tile([128, 1], BF16)
            nc.gpsimd.dma_start(out=battnqk, in_=battnqk_d.ap())
            nc.gpsimd.dma_start(
                out=bvb, in_=bvb_d.ap().rearrange("p (g n) -> p g n", n=64))
            nc.gpsimd.dma_start(out=idb, in_=idb_d.ap())
            nc.gpsimd.dma_start(out=maskb, in_=mask_d.ap())
            nc.gpsimd.dma_start(out=onesrow, in_=onesrow_d.ap())
            nc.gpsimd.dma_start(out=onesb, in_=onesb_d.ap())

            for rep in range(reps):
                x8 = xpool.tile([128, 4, 2, T], FP8, tag="x8", name=f"x8_{rep}")
                xT = xpool.tile([128, CT, T], BF16, tag="xT", name=f"xT_{rep}")
                for ctd in range(4):
                    if rep == 0:
                        nc.gpsimd.dma_start(
                            out=wqk8[:, ctd], in_=wqk8_d.ap().rearrange(
                                "(ctd p) (ko col m) -> p ctd ko col m",
                                p=128, ko=2, m=128)[:, ctd])
                    nc.sync.dma_start(
                        out=x8[:, ctd], in_=x8_d.ap().rearrange(
                            "(ctd p) (ko t) -> p ctd ko t",
                            p=128, ko=2)[:, ctd])
                for ct in range(CT):
                    nc.sync.dma_start(
                        out=xT[:, ct, :],
                        in_=xT_d.ap()[ct * 128:(ct + 1) * 128, :])
                if rep == 0:
                    nc.sync.dma_start(out=wv, in_=wv_d.ap().rearrange(
                        "(ct p) n -> p ct n", p=128))
                    nc.sync.dma_start(out=wproj, in_=wproj_d.ap().rearrange(
                        "(pr p) (oh n) -> p pr oh n", p=128, n=512))

                # ---- V pass: V_sb[keys, g=(kb,pair,side), 0:64]=V, [..,64]=1
                V_sb = vpool.tile([128, KB * 8, 65], BF16, tag="V",
                                  name=f"V_{rep}")
                nc.vector.tensor_copy(
                    out=V_sb[:, :, 64:65],
                    in_=onesb.to_broadcast([128, KB * 8, 1]))
                for tt in range(KB):
                    vac = ps.tile([128, 8, 64], F32, tag="mm", bufs=2,
                                  name=f"vac_{rep}_{tt}")
                    for ct in range(CT):
                        nc.tensor.matmul(vac, xT[:, ct, ts(tt, 128)],
                                         wv[:, ct, :],
                                         start=(ct == 0), stop=(ct == CT - 1),
                                         skip_group_check=True)
                    nc.vector.tensor_add(out=V_sb[:, tt * 8:(tt + 1) * 8, 0:64],
                                         in0=vac, in1=bvb)

                yTs = []
                for pr in range(NPAIR):
                    # ---- QK projection for this pair ----
                    qT = qkpool.tile([128, T], BF16, tag="qT", name=f"qT_{rep}_{pr}")
                    kT = qkpool.tile([128, T], BF16, tag="kT", name=f"kT_{rep}_{pr}")
                    for mt, dest in ((0, qT), (1, kT)):
                        for th in range(2):
                            acc = ps.tile([128, 512], F32, tag="mm", bufs=2,
                                          name=f"qk_{rep}_{pr}_{mt}_{th}")
                            for ctd in range(4):
                                nc.tensor.matmul(
                                    acc, wqk8[:, ctd, :, pr * 2 + mt, :],
                                    x8[:, ctd, :, ts(th, 512)],
                                    perf_mode=DR,
                                    start=(ctd == 0), stop=(ctd == 3),
                                    skip_group_check=True)
                            nc.scalar.activation(
                                out=dest[:, ts(th, 512)], in_=acc,
                                func=AF.Identity,
                                bias=battnqk[:, pr * 2 + mt:pr * 2 + mt + 1],
                                scale=1.0 / WSCALE)

                    # ---- causal attention, 2 heads row-packed ----
                    yT = ypool.tile([128, T], BF16, tag="yT",
                                    name=f"yT_{rep}_{pr}")
                    yTs.append(yT)
                    for qc in range(QC):
                        osA = ps.tile([65, 512], F32, tag="os", bufs=2,
                                      name=f"osA_{rep}_{pr}_{qc}")
                        osB = ps.tile([65, 512], F32, tag="os", bufs=2,
                                      name=f"osB_{rep}_{pr}_{qc}")
                        kb_max = 4 * (qc + 1)
                        for kb in range(kb_max):
                            qoff = max(0, kb * 128 - qc * 512)
                            first = kb == 0
                            last = kb == kb_max - 1
                            s2 = ps.tile([128, 2, 512], F32, tag="s2", bufs=2,
                                         name=f"s2_{rep}_{pr}_{qc}_{kb}")
                            qs = slice(qc * 512 + qoff, (qc + 1) * 512)
                            nc.tensor.matmul(s2[:, 0, qoff:512],
                                             kT[0:64, ts(kb, 128)],
                                             qT[0:64, qs], start=True, stop=False,
                                             tile_position=(0, 0),
                                             skip_group_check=True)
                            nc.tensor.matmul(s2[:, 1, qoff:512],
                                             kT[64:128, ts(kb, 128)],
                                             qT[64:128, qs], start=True, stop=False,
                                             tile_position=(64, 0),
                                             skip_group_check=True)
                            if kb * 128 >= qc * 512:   # diagonal block
                                nc.tensor.matmul(s2[:, 0, qoff:qoff + 128],
                                                 idb, maskb,
                                                 start=False, stop=True,
                                                 skip_group_check=True)
                                nc.tensor.matmul(s2[:, 1, qoff:qoff + 128],
                                                 idb, maskb,
                                                 start=False, stop=True,
                                                 skip_group_check=True)
                            pT = ppool.tile([128, 2, 512], BF16, tag="pT",
                                            name=f"pT_{rep}_{pr}_{qc}_{kb}")
                            nc.scalar.activation(out=pT[:, :, qoff:512],
                                                 in_=s2[:, :, qoff:512],
                                                 func=AF.Exp, scale=0.125)
                            nc.tensor.matmul(osA[:, qoff:512],
                                             V_sb[:, kb * 8 + pr * 2, :],
                                             pT[:, 0, qoff:512], start=first,
                                             stop=last, skip_group_check=True)
                            nc.tensor.matmul(osB[:, qoff:512],
                                             V_sb[:, kb * 8 + pr * 2 + 1, :],
                                             pT[:, 1, qoff:512], start=first,
                                             stop=last, skip_group_check=True)
                        recA = ppool.tile([1, 512], BF16, tag="rec", bufs=4,
                                          name=f"recA_{rep}_{pr}_{qc}")
                        recB = ppool.tile([1, 512], BF16, tag="rec", bufs=4,
                                          name=f"recB_{rep}_{pr}_{qc}")
                        with nc.allow_low_precision(reason="f32r softmax denom"):
                            nc.vector.reciprocal(out=recA, in_=osA[64:65, :])
                            nc.vector.reciprocal(out=recB, in_=osB[64:65, :])
                        bc = ps.tile([128, 512], F32, tag="mm", bufs=2,
                                     name=f"bc_{rep}_{pr}_{qc}")
                        nc.tensor.matmul(bc[0:64, :], onesrow, recA,
                                         start=True, stop=True,
                                         skip_group_check=True)
                        nc.tensor.matmul(bc[64:128, :], onesrow, recB,
                                         start=True, stop=True,
                                         skip_group_check=True)
                        bcsb = ppool.tile([128, 512], F32, tag="bcsb", bufs=2,
                                          name=f"bcsb_{rep}_{pr}_{qc}")
                        nc.vector.tensor_copy(out=bcsb, in_=bc)
                        nc.vector.tensor_mul(yT[0:64, ts(qc, 512)],
                                             osA[0:64, :], bcsb[0:64, :])
                        nc.vector.tensor_mul(yT[64:128, ts(qc, 512)],
                                             osB[0:64, :], bcsb[64:128, :])

                        # once every pair finished a token chunk, project it
                        if pr == NPAIR - 1:
                            for tt in range(4 * qc, 4 * qc + 4):
                                for oh in range(2):
                                    pp = ps.tile([128, 512], F32, tag="mm",
                                                 bufs=2, name=f"pp_{rep}_{tt}_{oh}")
                                    for pj in range(NPAIR):
                                        nc.tensor.matmul(
                                            pp, yTs[pj][:, ts(tt, 128)],
                                            wproj[:, pj, oh, :],
                                            start=(pj == 0), stop=(pj == NPAIR - 1),
                                            skip_group_check=True)
                                    ot = opool.tile([128, 512], F32, tag="ot",
                                                    name=f"ot_{rep}_{tt}_{oh}")
                                    if (tt + oh) % 2 == 0:
                                        nc.vector.tensor_copy(out=ot, in_=pp)
                                    else:
                                        nc.scalar.copy(out=ot, in_=pp)
                                    nc.gpsimd.dma_start(
                                        out=out_d.ap()[tt * 128:(tt + 1) * 128,
                                                       ts(oh, 512)],
                                        in_=ot)
    _split_sync_waits(nc)
    return nc


def make_in_maps(x, w_attn, b_attn, w_proj):
    import ml_dtypes
    bf16 = ml_dtypes.bfloat16

    fp8 = ml_dtypes.float8_e4m3fn

    def dr_rows(a):
        """[C, N] -> fp8 DoubleRow layout [C//2, 2*N]: row (ctd*128+ki),
        col (ko*N+n) holds a[ctd*256 + ki*2 + ko, n]."""
        cdim, n = a.shape
        v = a.reshape(cdim // 256, 128, 2, n)            # ctd, ki, ko, n
        v = np.ascontiguousarray(v.transpose(0, 1, 2, 3))
        return np.clip(v, -240.0, 240.0).astype(fp8).reshape(
            cdim // 2, 2 * n)

    xT = np.asarray(x, dtype=np.float32).reshape(B * T, C).T       # [C, B*T]
    xslices = [np.ascontiguousarray(xT[:, b * T:(b + 1) * T]) for b in range(B)]
    x8b = [dr_rows(a) for a in xslices]
    xTb = [a.astype(bf16) for a in xslices]
    idb = np.eye(128, dtype=np.float32).astype(bf16)
    maskb = np.tril(np.full((128, 128), NEG, dtype=np.float32), -1).astype(bf16)
    onesrow = np.ones((1, 64), dtype=np.float32).astype(bf16)
    onesb = np.ones((128, 1), dtype=np.float32).astype(bf16)

    w_attn = np.asarray(w_attn, dtype=np.float32)
    b_attn = np.asarray(b_attn, dtype=np.float32)
    w_proj = np.asarray(w_proj, dtype=np.float32)

    in_maps = []
    for c in range(NCORES):
        bi, hg = divmod(c, 2)
        qk_blocks, qk_bias, v_blocks, v_bias, wp = [], [], [], [], []
        for pr in range(NPAIR):
            h0 = (hg * 8 + pr * 2) * D
            qk_blocks += [w_attn[:, h0:h0 + 128],
                          w_attn[:, C + h0:C + h0 + 128]]
            qk_bias += [b_attn[h0:h0 + 128], b_attn[C + h0:C + h0 + 128]]
            v_blocks.append(w_attn[:, 2 * C + h0:2 * C + h0 + 128])
            v_bias.append(b_attn[2 * C + h0:2 * C + h0 + 128])
            wp.append(w_proj[h0:h0 + 128, :])
        wqk8 = dr_rows(np.concatenate(qk_blocks, axis=1) * WSCALE)
        battnqk = np.stack(qk_bias, axis=1).astype(np.float32)   # [128, 8]
        wv = np.ascontiguousarray(
            np.concatenate(v_blocks, axis=1)).astype(bf16)       # [C, 512]
        bvb = np.tile(np.concatenate(v_bias)[None, :],
                      (128, 1)).astype(bf16)                     # [128, 512]
        wprojc = np.ascontiguousarray(
            np.concatenate(wp, axis=0)).astype(bf16)             # [512, C]
        in_maps.append({
            "x8": x8b[bi], "wqk8": wqk8, "xTb": xTb[bi], "battnqk": battnqk,
            "wv": wv, "bvb": bvb, "wproj": wprojc,
            "idb": idb, "mask": maskb, "onesrow": onesrow, "onesb": onesb,
        })
    return in_maps


def kernel(x, w_attn, b_attn, w_proj, b_proj):
    x = np.asarray(x)
    w_attn = np.asarray(w_attn)
    b_attn = np.asarray(b_attn)
    w_proj = np.asarray(w_proj)
    b_proj = np.asarray(b_proj)

    if "nc" not in _nc_cache:
        _nc_cache["nc"] = build_nc()
    nc = _nc_cache["nc"]
    in_maps = make_in_maps(x, w_attn, b_attn, w_proj)

    res = run_bass_kernel_spmd(nc, in_maps, core_ids=list(range(NCORES)))
    out = np.empty((B, T, C), dtype=np.float32)
    for bi in range(B):
        out[bi] = res.results[2 * bi]["partial"]
        out[bi] += res.results[2 * bi + 1]["partial"]
        out[bi] += b_proj.astype(np.float32)
    return out
